# revision 17
# baseline (speedup 1.0000x reference)
"""GCN layer on 8 Trainium2 NeuronCores (Bass/Tile).

  H = X @ W^T + b                        [N, 128]
  out[r] = sum_{e: row[e]=r} val[e] * H[col[e]]

Sharding: nodes (rows of X and out) split 8 ways; W/b replicated; edges
partitioned by destination shard so the segment-sum is local to a core.
Each core computes its H shard (bf16), an AllGather builds the full H
table in DRAM, then a descriptor-generated DMA gather fetches H[col]
per edge (128-edge chunks, edges land on SBUF partitions).  The
segment-sum is performed on the tensor engine: for each chunk a
val-scaled one-hot matrix S^T[e, d] = val_e * (d == dest_e) is built on
the vector engine (iota compare) and  out_tile += S^T.T @ gathered
accumulates in PSUM per 128-row destination tile.
"""

import os

import numpy as np
import ml_dtypes

N_NODES = 100000
F_IN = 256
F_OUT = 128
N_CORES = 8
PAD_N = 100352          # 784 tiles of 128
CORE_ROWS = PAD_N // N_CORES      # 12544
TILES = CORE_ROWS // 128          # 98 dest tiles per core
BLK = 32768             # int16 gather index reach (rows per source block)
N_BLK = 4               # ceil(PAD_N / BLK): 3 full + 1 tail (2048 rows)
TGRP = 6                # dest tiles per segment group (= one PSUM bank each;
                        # a start=True matmul clears has_written for its WHOLE
                        # bank, so open accumulation groups must not share one)
N_TGRP = (TILES + TGRP - 1) // TGRP   # 17
SEG_CAP = 8             # max 128-edge chunks (1024 idxs) per dma_gather

BF16 = ml_dtypes.bfloat16

# AllGather is split into K slices (tiles per shard-slice) so gathers can
# begin as soon as the early slices land.  H_full is laid out slice-major:
# node (c, t, p) with t in slice k sits at
#   base_k + c*rows_k + (t - t0_k)*128 + p
AG_SLICES = [32, 32, 32, 2]           # tiles per slice: 8*rows_k == BLK
                                      # so gather window k == slice region k
_t0 = np.cumsum([0] + AG_SLICES[:-1])            # first tile of each slice
_rows = np.asarray(AG_SLICES) * 128              # rows per core per slice
_base = np.concatenate([[0], np.cumsum(_rows * N_CORES)[:-1]])


def _hfull_pos(n):
    """Map node id -> position in the slice-major H_full layout."""
    c = n // CORE_ROWS
    r = n % CORE_ROWS
    t = r // 128
    k = np.searchsorted(np.cumsum(AG_SLICES), t, side="right")
    return _base[k] + c * _rows[k] + (t - _t0[k]) * 128 + (r % 128)


def _prep_edges(edge_row, edge_col, edge_val):
    """Partition edges by (dest core, dest tile, src block); pad each
    (tile, block) group to a shared chunk count so all 8 cores run one
    SPMD program.  Returns per-core packed arrays + the shared schedule."""
    core = edge_row // CORE_ROWS
    t = (edge_row % CORE_ROWS) // 128
    d_loc = (edge_row % 128).astype(np.float32)
    pos = _hfull_pos(edge_col)
    b = pos // BLK
    idx_loc = (pos % BLK).astype(np.int16)

    # group key per edge within its core: t * N_BLK + b
    gkey = (t * N_BLK + b).astype(np.int64)
    n_groups = TILES * N_BLK
    cnt = np.zeros((N_CORES, n_groups), np.int64)
    for c in range(N_CORES):
        cnt[c] = np.bincount(gkey[core == c], minlength=n_groups)
    cnt_max = cnt.max(axis=0)
    C = -(-cnt_max // 128)          # chunks per (t, b), shared across cores

    # schedule: tile-groups of TGRP tiles, block-minor inside the group
    # chunk_base[t, b] = first global chunk id of that (t, b)
    chunk_base = np.full(n_groups, -1, np.int64)
    segments = []   # (b, blk_rows, n_seg_chunks, [(t, k chunks)...], chunk0)
    chunks = []     # per chunk: (tile t, start, stop)
    nxt = 0
    for tg in range(N_TGRP):
        tiles = range(tg * TGRP, min(TILES, (tg + 1) * TGRP))
        # first/last (t, b) with chunks, for PSUM start/stop flags
        first_g = {}
        last_g = {}
        for tt in tiles:
            gs = [g for g in range(tt * N_BLK, tt * N_BLK + N_BLK) if C[g] > 0]
            if gs:
                first_g[tt] = gs[0]
                last_g[tt] = gs[-1]
        for bb in range(N_BLK):
            c0 = nxt
            for tt in tiles:
                g = tt * N_BLK + bb
                if C[g] == 0:
                    continue
                chunk_base[g] = nxt
                for k in range(C[g]):
                    start = (g == first_g[tt]) and (k == 0)
                    stop = (g == last_g[tt]) and (k == C[g] - 1)
                    chunks.append((tt, start, stop))
                nxt += C[g]
            if nxt > c0:
                lo = bb * BLK
                hi = min(lo + BLK, PAD_N)
                # dma_gather wedges the device above 1024 indices/call:
                # split the (tile-group, block) range into <=SEG_CAP chunks
                for p0 in range(c0, nxt, SEG_CAP):
                    segments.append((lo, hi, min(SEG_CAP, nxt - p0), p0))
    n_chunks = nxt
    n_tok = n_chunks * 128

    # pack per-core token arrays
    per_core = []
    tok_base = chunk_base * 128
    for c in range(N_CORES):
        m = core == c
        gk = gkey[m]
        order = np.argsort(gk, kind="stable")
        gks = gk[order]
        # rank within group
        grp_start = np.searchsorted(gks, np.arange(n_groups), side="left")
        rank = np.arange(len(gks)) - grp_start[gks]
        slot = tok_base[gks] + rank
        idx_tok = np.zeros(n_tok, np.int16)
        val_tok = np.zeros(n_tok, np.float32)
        dest_tok = np.zeros(n_tok, np.float32)
        idx_tok[slot] = idx_loc[m][order]
        val_tok[slot] = edge_val[m][order]
        dest_tok[slot] = d_loc[m][order]
        # gather index layout: token i -> [i % 16, i // 16], replicated
        # across the 8 gpsimd cores (8 x 16 = 128 partitions)
        idx16 = idx_tok.reshape(-1, 16).T
        idx_packed = np.tile(idx16, (8, 1)).copy()
        val_arr = val_tok.reshape(n_chunks, 128).T.astype(np.float32).copy()
        dest_arr = dest_tok.reshape(n_chunks, 128).T.astype(np.float32).copy()
        per_core.append((idx_packed, val_arr, dest_arr))
    return per_core, segments, chunks, n_chunks


def _build_program(segments, chunks, n_chunks):
    import concourse.bacc as bacc
    import concourse.bass as bass
    import concourse.mybir as mybir
    import concourse.tile as tile

    DT16 = mybir.dt.bfloat16
    DT32 = mybir.dt.float32
    nc = bacc.Bacc(None, target_bir_lowering=False, debug=False)

    xt0 = nc.declare_dram_parameter("XT0", [128, CORE_ROWS], DT16, isOutput=False)
    xt1 = nc.declare_dram_parameter("XT1", [128, CORE_ROWS], DT16, isOutput=False)
    wt0 = nc.declare_dram_parameter("WT0", [128, F_OUT], DT16, isOutput=False)
    wt1 = nc.declare_dram_parameter("WT1", [128, F_OUT], DT16, isOutput=False)
    brow = nc.declare_dram_parameter("BROW", [1, F_OUT], DT16, isOutput=False)
    ones = nc.declare_dram_parameter("ONES", [1, 128], DT16, isOutput=False)
    iota_in = nc.declare_dram_parameter("IOTA", [128, 128], DT16, isOutput=False)
    idx_in = nc.declare_dram_parameter(
        "IDX", [128, n_chunks * 8], mybir.dt.int16, isOutput=False)
    val_in = nc.declare_dram_parameter("VAL", [128, n_chunks], DT32, isOutput=False)
    dest_in = nc.declare_dram_parameter("DEST", [128, n_chunks], DT32, isOutput=False)
    out_ext = nc.declare_dram_parameter("OUT", [CORE_ROWS, F_OUT], DT32, isOutput=True)

    with tile.TileContext(nc) as tc:
        with (
            tc.tile_pool(name="dram", bufs=1, space="DRAM") as dram,
            tc.tile_pool(name="const", bufs=1) as constp,
            tc.tile_pool(name="hstage", bufs=3) as hpool,
            tc.tile_pool(name="gat", bufs=3) as gpool,
            tc.tile_pool(name="st", bufs=4) as stpool,
            tc.tile_pool(name="ost", bufs=2) as opool,
            tc.tile_pool(name="pgemm", bufs=2, space=bass.MemorySpace.PSUM) as pg,
            tc.tile_pool(name="pseg", bufs=6, space=bass.MemorySpace.PSUM) as ps,
        ):
            h_shard = dram.tile([CORE_ROWS, F_OUT], DT16)
            h_full = dram.tile([PAD_N, F_OUT], DT16)

            # ---- constants / inputs to SBUF -------------------------------
            idx_sb = constp.tile([128, n_chunks * 8], mybir.dt.int16)
            val_sb = constp.tile([128, n_chunks], DT32)
            dest_sb = constp.tile([128, n_chunks], DT32)
            nc.scalar.dma_start(idx_sb[:], idx_in[:, :])
            nc.scalar.dma_start(val_sb[:], val_in[:, :])
            nc.scalar.dma_start(dest_sb[:], dest_in[:, :])
            xt0_sb = constp.tile([128, CORE_ROWS], DT16)
            xt1_sb = constp.tile([128, CORE_ROWS], DT16)
            for _k in range(len(AG_SLICES)):
                _a = int(_t0[_k]) * 128
                _b = _a + AG_SLICES[_k] * 128
                nc.scalar.dma_start(xt0_sb[:, _a:_b], xt0[:, _a:_b])
                nc.scalar.dma_start(xt1_sb[:, _a:_b], xt1[:, _a:_b])
            wt0_sb = constp.tile([128, F_OUT], DT16)
            wt1_sb = constp.tile([128, F_OUT], DT16)
            brow_sb = constp.tile([1, F_OUT], DT16)
            ones_sb = constp.tile([1, 128], DT16)
            iota_sb = constp.tile([128, 128], DT16)
            nc.scalar.dma_start(wt0_sb[:], wt0[:, :])
            nc.scalar.dma_start(wt1_sb[:], wt1[:, :])
            nc.scalar.dma_start(brow_sb[:], brow[:, :])
            nc.scalar.dma_start(ones_sb[:], ones[:, :])
            nc.scalar.dma_start(iota_sb[:], iota_in[:, :])

            # ---- phase 1: H shard GEMM, sliced all-gathers ----------------
            # The all-gather is split into len(AG_SLICES) collectives over
            # tile ranges of the shard; gathers depending only on early
            # slices of h_full can start before the whole table lands.
            hs_v = h_shard[:].rearrange("(n p) f -> p n f", p=128)
            HB = 9   # max tiles per H write batch
            PB = 4   # node tiles per PSUM bank
            for k, ntiles in enumerate(AG_SLICES):
                t0 = int(_t0[k])
                for i0 in range(t0, t0 + ntiles, HB):
                    nb = min(HB, t0 + ntiles - i0)
                    h_sb = hpool.tile([128, nb, F_OUT], DT16)
                    for j0 in range(0, nb, PB):
                        pb = min(PB, nb - j0)
                        acc = pg.tile([128, pb, F_OUT], DT32)
                        for j in range(j0, j0 + pb):
                            i = i0 + j
                            a = acc[:, j - j0, :]
                            nc.tensor.matmul(
                                a, xt0_sb[:, i * 128:(i + 1) * 128], wt0_sb[:],
                                start=True, stop=False)
                            nc.tensor.matmul(
                                a, xt1_sb[:, i * 128:(i + 1) * 128], wt1_sb[:],
                                start=False, stop=False)
                            nc.tensor.matmul(
                                a, ones_sb[:], brow_sb[:],
                                start=False, stop=True)
                        nc.scalar.copy(h_sb[:, j0:j0 + pb, :], acc[:])
                    nc.sync.dma_start(hs_v[:, i0:i0 + nb, :], h_sb[:])
                nc.gpsimd.collective_compute(
                    "AllGather",
                    mybir.AluOpType.bypass,
                    replica_groups=[list(range(N_CORES))],
                    ins=[h_shard[t0 * 128:(t0 + ntiles) * 128, :].opt()],
                    outs=[h_full[int(_base[k]):int(_base[k])
                                 + N_CORES * int(_rows[k]), :].opt()],
                )

            # ---- phase 2: gather + S^T matmul segment-sum ----------------
            out_v = out_ext.rearrange("(n p) f -> p n f", p=128)
            grp_state = {}    # tg -> [osb, ntg, n_stops_left]
            pacc_by_tile = {}
            for (lo, hi, n_seg, c0) in segments:
                g = gpool.tile([128, n_seg, F_OUT], DT16)
                nc.gpsimd.dma_gather(
                    out_ap=g[:, :, :],
                    in_ap=h_full[lo:hi, :],
                    idxs_ap=idx_sb[:, c0 * 8:(c0 + n_seg) * 8],
                    num_idxs=n_seg * 128,
                    num_idxs_reg=n_seg * 128,
                    elem_size=F_OUT,
                    single_packet=False,
                )
                for k in range(n_seg):
                    ci = c0 + k
                    tt, st_flag, sp_flag = chunks[ci]
                    tg = tt // TGRP
                    if tg not in grp_state:
                        ntg = min(TILES, (tg + 1) * TGRP) - tg * TGRP
                        grp_state[tg] = [
                            opool.tile([128, ntg, F_OUT], DT32, name='osb'),
                            ntg, ntg]
                    if st_flag:
                        # one accumulator per PSUM bank (whole-bank
                        # has_written clear on start=True)
                        pacc_by_tile[tt] = ps.tile(
                            [128, F_OUT], DT32, name='pacc')
                    osb, ntg, _ = grp_state[tg]
                    tl = tt - tg * TGRP
                    st_t = stpool.tile([128, 128], DT16)
                    nc.vector.tensor_scalar(
                        out=st_t[:], in0=iota_sb[:],
                        scalar1=dest_sb[:, ci:ci + 1],
                        scalar2=val_sb[:, ci:ci + 1],
                        op0=mybir.AluOpType.is_equal,
                        op1=mybir.AluOpType.mult,
                    )
                    nc.tensor.matmul(
                        pacc_by_tile[tt][:], st_t[:], g[:, k, :],
                        start=st_flag, stop=sp_flag)
                    if sp_flag:
                        nc.scalar.copy(osb[:, tl, :], pacc_by_tile[tt][:])
                        del pacc_by_tile[tt]
                        grp_state[tg][2] -= 1
                        if grp_state[tg][2] == 0:
                            nc.sync.dma_start(
                                out_v[:, tg * TGRP:tg * TGRP + ntg, :],
                                osb[:])
                            del grp_state[tg]

    nc.compile()
    return nc


_CACHE = {}


def kernel(X, edge_row, edge_col, edge_val, W, b):
    from concourse.bass_utils import run_bass_kernel_spmd

    X = np.asarray(X, np.float32)
    W = np.asarray(W, np.float32)
    b = np.asarray(b, np.float32)
    edge_row = np.asarray(edge_row, np.int64)
    edge_col = np.asarray(edge_col, np.int64)
    edge_val = np.asarray(edge_val, np.float32)

    per_core, segments, chunks, n_chunks = _prep_edges(
        edge_row, edge_col, edge_val)

    key = ("v1", n_chunks, tuple(s[3] for s in segments))
    if key not in _CACHE:
        _CACHE[key] = _build_program(segments, chunks, n_chunks)
    nc = _CACHE[key]

    # host-side tensor prep (shared)
    Xp = np.zeros((PAD_N, F_IN), np.float32)
    Xp[:N_NODES] = X
    XT = np.ascontiguousarray(Xp.T).astype(BF16)      # [256, PAD_N]
    WT = np.ascontiguousarray(W.T).astype(BF16)       # [256, 128]
    brow = b.reshape(1, F_OUT).astype(BF16)
    ones = np.ones((1, 128), BF16)
    iota = np.tile(np.arange(128, dtype=np.float32), (128, 1)).astype(BF16)

    in_maps = []
    for c in range(N_CORES):
        idx_packed, val_arr, dest_arr = per_core[c]
        sl = slice(c * CORE_ROWS, (c + 1) * CORE_ROWS)
        in_maps.append({
            "XT0": np.ascontiguousarray(XT[:128, sl]),
            "XT1": np.ascontiguousarray(XT[128:, sl]),
            "WT0": np.ascontiguousarray(WT[:128]),
            "WT1": np.ascontiguousarray(WT[128:]),
            "BROW": brow,
            "ONES": ones,
            "IOTA": iota,
            "IDX": idx_packed,
            "VAL": val_arr,
            "DEST": dest_arr,
        })

    trace = bool(os.environ.get("GCN_KERNEL_TRACE"))
    kw = {}
    if trace:
        kw = dict(trace=True, tmpdir=os.environ.get("GCN_KERNEL_TMPDIR"))
    res = run_bass_kernel_spmd(nc, in_maps, list(range(N_CORES)), **kw)
    if trace:
        kernel.last_results = res

    out = np.concatenate(
        [res.results[c]["OUT"] for c in range(N_CORES)], axis=0)
    return np.ascontiguousarray(out[:N_NODES])
